# revision 33
# baseline (speedup 1.0000x reference)
"""Trainium2 Bass kernel for nn_AGNN (meta-GNN message passing, G=2, B=16, N=128, D=128).

Sharding: data-parallel over meta-batch B across 8 NeuronCores (2 tasks/core).
All compute per task is local; no collectives.

The pairwise-difference MLP (the dominant cost, 5 fp32 matmul passes over all
N^2 node pairs) exploits s(i,j)=s(j,i) symmetry: only pairs with j >= 4*(i//4)
are computed (~52%), and the full matrix is rebuilt as SU + SU^T with a
strict-upper mask. BatchNorm folds into the activation's per-partition
scale/bias; leaky-relu is the Prelu activation (alpha=0.01); sigmoid is
0.5+0.5*tanh(x/2) to stay in one activation table set; node_sim uses the Gram
trick (-|vi-vj|^2 = 2G - |vi|^2 - |vj|^2) in bf16.

Self-contained: builds + compiles the Bass graph on first call, then runs via
run_bass_kernel_spmd on cores 0-7 and reassembles full outputs.
"""

import math
import numpy as np

import concourse.bacc as bacc
import concourse.mybir as mybir
from concourse.bass_utils import run_bass_kernel_spmd
from concourse.tile import TileContext
from concourse.masks import make_identity

F32 = mybir.dt.float32
BF16 = mybir.dt.bfloat16
U32 = mybir.dt.uint32
I32 = mybir.dt.int32
AF = mybir.ActivationFunctionType
ALU = mybir.AluOpType

G = 2
B = 16
N = 128
D = 128
BASE = 128
H = 8
DK = 16
BN_EPS = 1e-5
BNS = 1.0 / math.sqrt(1.0 + BN_EPS)
NCORES = 8
TASKS_PER_CORE = B // NCORES
QCH = 4  # i-rows per quad

# Symmetric chunking: quad q covers i in [4q,4q+4), j in [4q,128).
# Quads are greedily grouped into chunks of total free-width <= 512.
_QUADS = [(4 * q, 4 * q, QCH * (N - 4 * q)) for q in range(N // QCH)]  # (i0, jmin, w)
CHUNKS = []
_cur, _curw = [], 0
for _qd in _QUADS:
    if _curw + _qd[2] > 512 and _cur:
        CHUNKS.append(_cur)
        _cur, _curw = [], 0
    _cur.append(_qd)
    _curw += _qd[2]
if _cur:
    CHUNKS.append(_cur)


def build_nc():
    nc = bacc.Bacc("TRN2", target_bir_lowering=False, debug=False, num_devices=NCORES)

    vp_d = nc.declare_dram_parameter("vp", [TASKS_PER_CORE, N, D], F32, isOutput=False)
    ep_d = nc.declare_dram_parameter("ep", [TASKS_PER_CORE, N, N], F32, isOutput=False)
    ps_w1 = nc.declare_dram_parameter("ps_w1", [G, 2 * BASE, D], F32, isOutput=False)
    ps_g1 = nc.declare_dram_parameter("ps_g1", [G, 2 * BASE], F32, isOutput=False)
    ps_b1 = nc.declare_dram_parameter("ps_b1", [G, 2 * BASE], F32, isOutput=False)
    ps_w2 = nc.declare_dram_parameter("ps_w2", [G, BASE, 2 * BASE], F32, isOutput=False)
    ps_g2 = nc.declare_dram_parameter("ps_g2", [G, BASE], F32, isOutput=False)
    ps_b2 = nc.declare_dram_parameter("ps_b2", [G, BASE], F32, isOutput=False)
    ps_w3 = nc.declare_dram_parameter("ps_w3", [G, 1, BASE], F32, isOutput=False)
    ps_b3 = nc.declare_dram_parameter("ps_b3", [G, 1], F32, isOutput=False)
    d2p_w1 = nc.declare_dram_parameter("d2p_w1", [G, 2 * BASE, 2 * D], F32, isOutput=False)
    d2p_g1 = nc.declare_dram_parameter("d2p_g1", [G, 2 * BASE], F32, isOutput=False)
    d2p_b1 = nc.declare_dram_parameter("d2p_b1", [G, 2 * BASE], F32, isOutput=False)
    d2p_w2 = nc.declare_dram_parameter("d2p_w2", [G, BASE, 2 * BASE], F32, isOutput=False)
    d2p_g2 = nc.declare_dram_parameter("d2p_g2", [G, BASE], F32, isOutput=False)
    d2p_b2 = nc.declare_dram_parameter("d2p_b2", [G, BASE], F32, isOutput=False)
    wq_d = nc.declare_dram_parameter("wq", [G, H * DK, D], F32, isOutput=False)
    wk_d = nc.declare_dram_parameter("wk", [G, H * DK, D], F32, isOutput=False)
    # out[t, 0:2] = eps g0/g1; out[t, 2:4] = sims g0/g1; out[t, 4] = final vp
    out_d = nc.declare_dram_parameter("out", [TASKS_PER_CORE, 5, N, N], F32, isOutput=True)

    with TileContext(nc) as tc:
        with (
            tc.tile_pool(name="consts", bufs=1) as cpool,
            tc.tile_pool(name="weights", bufs=1) as wpool,
            tc.tile_pool(name="sb", bufs=3) as sb,
            tc.tile_pool(name="sb3", bufs=3) as sb3,
            tc.tile_pool(name="state", bufs=1) as st,
            tc.tile_pool(name="psA", bufs=4, space="PSUM") as psA,
            tc.tile_pool(name="psB", bufs=2, space="PSUM") as psB,
            tc.tile_pool(name="psC", bufs=1, space="PSUM") as psC,
            tc.tile_pool(name="psM", bufs=1, space="PSUM") as psM,
            tc.tile_pool(name="dsc", bufs=2, space="DRAM") as dsc,
        ):
            # ================= constants =================
            ident = cpool.tile([N, N], F32)
            make_identity(nc, ident[:])

            iota_i32 = cpool.tile([N, N], I32)
            nc.gpsimd.iota(iota_i32[:], pattern=[[1, N]], base=0, channel_multiplier=0)
            iota_bf = cpool.tile([N, N], BF16)
            nc.vector.tensor_copy(iota_bf[:], iota_i32[:])

            # 1.0 off-diagonal, 0 diagonal
            offd = cpool.tile([N, N], F32)
            nc.gpsimd.memset(offd[:], 1.0)
            nc.gpsimd.affine_select(out=offd[:], in_=offd[:], compare_op=ALU.not_equal,
                                    fill=0.0, base=0, pattern=[[-1, N]], channel_multiplier=1)
            # 1e-6 everywhere + 1.0 extra on diagonal
            eyep = cpool.tile([N, N], F32)
            nc.gpsimd.memset(eyep[:], 1e-6)
            nc.gpsimd.affine_select(out=eyep[:], in_=eyep[:], compare_op=ALU.not_equal,
                                    fill=1.0 + 1e-6, base=0, pattern=[[-1, N]], channel_multiplier=1)
            # 0.5 strictly above diagonal, 0 elsewhere  (j - i > 0)
            suh = cpool.tile([N, N], F32)
            nc.gpsimd.memset(suh[:], 0.5)
            nc.gpsimd.affine_select(out=suh[:], in_=suh[:], compare_op=ALU.is_gt,
                                    fill=0.0, base=0, pattern=[[1, N]], channel_multiplier=-1)

            zeros_nn = cpool.tile([N, N], F32)
            nc.gpsimd.memset(zeros_nn[:], 0.0)

            ones_col_bf = cpool.tile([N, 1], BF16)
            nc.vector.memset(ones_col_bf[:], 1.0)
            ones_col_f = cpool.tile([N, 1], F32)
            nc.vector.memset(ones_col_f[:], 1.0)
            ones_row_f = cpool.tile([1, N], F32)
            nc.vector.memset(ones_row_f[:], 1.0)

            # ================= weight prep =================
            _wn = [0]

            def transpose_from_dram(dram_ap, name_tag):
                _wn[0] += 1
                tmp = sb3.tile([N, N], F32, tag="wtmp")
                nc.gpsimd.dma_start(out=tmp[:], in_=dram_ap)
                tps = psA.tile([N, N], F32, tag="z1")
                nc.tensor.transpose(tps[:], tmp[:], ident[:])
                dst = wpool.tile([N, N], F32, tag=name_tag)
                nc.scalar.copy(dst[:], tps[:])
                return dst

            # load task inputs FIRST so they aren't queued behind weight DMAs
            VP, EP, VPT = {}, {}, {}
            for b in range(TASKS_PER_CORE):
                vp_cur = st.tile([N, D], F32, tag=f"vp{b}")
                ep_cur = st.tile([N, N], F32, tag=f"ep{b}")
                vpT = st.tile([N, N], F32, tag=f"vpT{b}")
                nc.sync.dma_start(out=vp_cur[:], in_=vp_d[b])
                nc.sync.dma_start(out=ep_cur[:], in_=ep_d[b])
                vpT_ps = psM.tile([N, N], F32, tag="misc")
                nc.tensor.transpose(vpT_ps[:], vp_cur[:], ident[:])
                nc.scalar.copy(vpT[:], vpT_ps[:])
                VP[b], EP[b], VPT[b] = vp_cur, ep_cur, vpT

            # all 1-D weight vectors for a gen are loaded as rows of one tile,
            # transposed once, and used as per-partition scale/bias columns.
            # rows 0..5: ps_g1, ps_b1, d2p_g1, d2p_b1 (256-wide), ps_g2, ps_b2
            # rows 6..11: d2p_g2, d2p_b2, w3, b3(bcast later) -- 128-wide
            VECT = []  # per gen: [128, 32] tile; column layout computed below
            for g in range(G):
                vrows = wpool.tile([16, 256], F32, tag=f"vrows{g}")
                dmae = nc.sync
                dmae.dma_start(out=vrows[0:1, :], in_=ps_g1[g:g + 1, :])
                dmae.dma_start(out=vrows[1:2, :], in_=ps_b1[g:g + 1, :])
                dmae.dma_start(out=vrows[2:3, :], in_=d2p_g1[g:g + 1, :])
                dmae.dma_start(out=vrows[3:4, :], in_=d2p_b1[g:g + 1, :])
                dmae.dma_start(out=vrows[4:5, 0:128], in_=ps_g2[g:g + 1, :])
                dmae.dma_start(out=vrows[5:6, 0:128], in_=ps_b2[g:g + 1, :])
                dmae.dma_start(out=vrows[6:7, 0:128], in_=d2p_g2[g:g + 1, :])
                dmae.dma_start(out=vrows[7:8, 0:128], in_=d2p_b2[g:g + 1, :])
                dmae.dma_start(out=vrows[8:9, 0:128], in_=ps_w3[g, 0:1, :])
                vt = wpool.tile([N, 32], F32, tag=f"vect{g}")
                for half in range(2):
                    tp = psM.tile([N, 16], F32, tag="misc")
                    nc.tensor.transpose(tp[:], vrows[:, half * 128:(half + 1) * 128],
                                        ident[0:16, 0:16])
                    nc.scalar.copy(vt[:, half * 16:(half + 1) * 16], tp[:])
                # fold 1/sqrt(1+eps) into the four gamma columns (g1, d2p_g1 both
                # halves; g2, d2p_g2 first half)
                nc.vector.tensor_scalar_mul(vt[:, 0:1], vt[:, 0:1], BNS)
                nc.vector.tensor_scalar_mul(vt[:, 2:3], vt[:, 2:3], BNS)
                nc.vector.tensor_scalar_mul(vt[:, 4:5], vt[:, 4:5], BNS)
                nc.vector.tensor_scalar_mul(vt[:, 6:7], vt[:, 6:7], BNS)
                nc.vector.tensor_scalar_mul(vt[:, 16:17], vt[:, 16:17], BNS)
                nc.vector.tensor_scalar_mul(vt[:, 18:19], vt[:, 18:19], BNS)
                VECT.append(vt)

            def colvec_for(g, row, half):
                return VECT[g][:, half * 16 + row:half * 16 + row + 1]

            W1T, G1, B1, W2T, G2, B2, W3 = [], [], [], [], [], [], []
            DW1T, DG1, DB1, DW2T, DG2, DB2, WQT, WKT, B3C = [], [], [], [], [], [], [], [], []
            for g in range(G):
                W1T.append([transpose_from_dram(ps_w1[g, ct * 128:(ct + 1) * 128, :], f"w1t{g}{ct}")
                            for ct in range(2)])
                G1.append([colvec_for(g, 0, ct) for ct in range(2)])
                B1.append([colvec_for(g, 1, ct) for ct in range(2)])
                W2T.append([transpose_from_dram(ps_w2[g, :, ct * 128:(ct + 1) * 128], f"w2t{g}{ct}")
                            for ct in range(2)])
                G2.append(colvec_for(g, 4, 0))
                B2.append(colvec_for(g, 5, 0))
                W3.append(colvec_for(g, 8, 0))
                DW1T.append([[transpose_from_dram(
                    d2p_w1[g, ct * 128:(ct + 1) * 128, kp * 128:(kp + 1) * 128], f"dw1t{g}{kp}{ct}")
                    for ct in range(2)] for kp in range(2)])
                DG1.append([colvec_for(g, 2, ct) for ct in range(2)])
                DB1.append([colvec_for(g, 3, ct) for ct in range(2)])
                DW2T.append([transpose_from_dram(d2p_w2[g, :, ct * 128:(ct + 1) * 128], f"dw2t{g}{ct}")
                             for ct in range(2)])
                DG2.append(colvec_for(g, 6, 0))
                DB2.append(colvec_for(g, 7, 0))
                WQT.append(transpose_from_dram(wq_d[g], f"wqt{g}"))
                WKT.append(transpose_from_dram(wk_d[g], f"wkt{g}"))
                b3c = wpool.tile([N, 1], F32, tag=f"b3c{g}")
                nc.sync.dma_start(out=b3c[:, 0:1],
                                  in_=ps_b3[g:g + 1, 0:1].to_broadcast((N, 1)))
                nc.scalar.mul(b3c[:, 0:1], b3c[:, 0:1], 0.5)
                B3C.append(b3c)

            # ================= per-task compute =================

            SFLAT = {}

            def mlp_phase(b, g):
                vp_cur, ep_cur, vpT = VP[b], EP[b], VPT[b]
                if True:
                    # ---------- sims (node_sim) via Gram trick, fp32 ----------
                    sqf = sb.tile([N, N], F32, tag="sqbf")
                    nc.scalar.activation(sqf[:], vpT[:], AF.Square)
                    nsq_ps = psM.tile([1, N], F32, tag="misc")
                    nc.tensor.matmul(nsq_ps[:], ones_col_f[:], sqf[:])
                    nsqn = sb.tile([1, N], F32, tag="nsqn")
                    nc.scalar.mul(nsqn[:], nsq_ps[:], -1.0)
                    vpT2 = sb.tile([N, N], F32, tag="vpT2b")
                    nc.scalar.mul(vpT2[:], vpT[:], 2.0)
                    nsim_ps = psM.tile([N, N], F32, tag="misc")
                    nc.tensor.matmul(nsim_ps[:], vpT2[:], vpT[:], start=True, stop=False)
                    nc.tensor.matmul(nsim_ps[:], nsqn[:], ones_row_f[:], start=False, stop=False)
                    nc.tensor.matmul(nsim_ps[:], ones_row_f[:], nsqn[:], start=False, stop=True)
                    nsim_sb = sb.tile([N, N], F32, tag="nsim")
                    nc.scalar.copy(nsim_sb[:], nsim_ps[:])
                    nc.sync.dma_start(out=out_d[b, 2 + g], in_=nsim_sb[:])

                    # ---------- pair MLP over the block-upper triangle ----------
                    s_flat = sb.tile([N, N], F32, tag=f"sflat{b}")
                    s_dram = dsc.tile([N, N], F32, tag="sdram")
                    nc.sync.dma_start(out=s_dram[:], in_=zeros_nn[:])

                    stash = {}

                    def stage_a(ic):
                        chunk = CHUNKS[ic]
                        F = sum(w for _, _, w in chunk)
                        dsub = sb3.tile([N, 512], F32, tag="dsub")
                        off = 0
                        for (i0, jmin, w) in chunk:
                            nj = N - jmin
                            bc_i = vpT[:, i0:i0 + QCH, None].to_broadcast((N, QCH, nj))
                            bc_j = vpT[:, None, jmin:N].to_broadcast((N, QCH, nj))
                            nc.vector.tensor_tensor(
                                out=dsub[:, off:off + w].rearrange("p (a c) -> p a c", a=QCH),
                                in0=bc_i, in1=bc_j, op=ALU.subtract)
                            off += w
                        dsq = sb3.tile([N, 512], F32, tag="dsq")
                        nc.vector.tensor_tensor(out=dsq[:, 0:F], in0=dsub[:, 0:F],
                                                in1=dsub[:, 0:F], op=ALU.mult)
                        h1 = []
                        for ct in range(2):
                            z1 = psA.tile([N, 512], F32, tag="z1")
                            nc.tensor.matmul(z1[:, 0:F], W1T[g][ct][:], dsq[:, 0:F])
                            h1t = sb3.tile([N, 512], F32, tag=f"h1{ct}")
                            nc.scalar.activation(h1t[:, 0:F], z1[:, 0:F], AF.Prelu,
                                                 bias=B1[g][ct], scale=G1[g][ct],
                                                 alpha=0.01)
                            h1.append(h1t)
                        stash[("h1", ic)] = (h1, F)

                    def stage_b(ic):
                        h1, F = stash.pop(("h1", ic))
                        z2 = psB.tile([N, 512], F32, tag="z2")
                        nc.tensor.matmul(z2[:, 0:F], W2T[g][0][:], h1[0][:, 0:F], start=True, stop=False)
                        nc.tensor.matmul(z2[:, 0:F], W2T[g][1][:], h1[1][:, 0:F], start=False, stop=True)
                        h2 = sb3.tile([N, 512], F32, tag="h2")
                        nc.scalar.activation(h2[:, 0:F], z2[:, 0:F], AF.Prelu,
                                             bias=B2[g], scale=G2[g], alpha=0.01)
                        stash[("h2", ic)] = (h2, F)

                    def stage_c(ic):
                        h2, F = stash.pop(("h2", ic))
                        chunk = CHUNKS[ic]
                        z3 = psC.tile([1, 512], F32, tag="z3")
                        nc.tensor.matmul(z3[:, 0:F], W3[g], h2[:, 0:F])
                        z3sb = sb3.tile([1, 512], F32, tag="z3sb")
                        if ic % 2 == 0:
                            nc.vector.tensor_copy(z3sb[:, 0:F], z3[:, 0:F])
                        else:
                            nc.scalar.copy(z3sb[:, 0:F], z3[:, 0:F])
                        off = 0
                        for iq, (i0, jmin, w) in enumerate(chunk):
                            srcq = z3sb[0:1, off:off + w].rearrange("a (b c) -> a b c", b=QCH)
                            dstq = s_dram[i0:i0 + QCH, jmin:N][None, :, :]
                            if iq % 2 == 0:
                                nc.sync.dma_start(out=dstq, in_=srcq)
                            else:
                                nc.gpsimd.dma_start(out=dstq, in_=srcq)
                            off += w

                    NCHK = len(CHUNKS)
                    for ic in range(NCHK + 2):
                        if ic < NCHK:
                            stage_a(ic)
                        if 1 <= ic:
                            if ic - 1 < NCHK:
                                stage_b(ic - 1)
                        if 2 <= ic:
                            stage_c(ic - 2)
                    nc.sync.dma_start(out=s_flat[:], in_=s_dram[:])
                    SFLAT[b] = s_flat
                return

            def post_phase(b, g):
                vp_cur, ep_cur, vpT = VP[b], EP[b], VPT[b]
                s_flat = SFLAT[b]
                if True:
                    # s(full) = SU + SU^T with SU = 0.5*(tanh+1) strictly above diag
                    tt = sb.tile([N, N], F32, tag="tt")
                    nc.scalar.activation(tt[:], s_flat[:], AF.Tanh,
                                         bias=B3C[g][:, 0:1], scale=0.5)
                    s_u = sb.tile([N, N], F32, tag="su")
                    nc.vector.scalar_tensor_tensor(out=s_u[:], in0=tt[:], scalar=1.0,
                                                   in1=suh[:], op0=ALU.add, op1=ALU.mult)
                    suT_ps = psM.tile([N, N], F32, tag="misc")
                    nc.tensor.transpose(suT_ps[:], s_u[:], ident[:])
                    s_full = sb.tile([N, N], F32, tag="sfull")
                    nc.vector.tensor_tensor(out=s_full[:], in0=s_u[:], in1=suT_ps[:],
                                            op=ALU.add)
                    # ep0 = ep * (1-eye); e = s_full * ep0
                    ep0f = sb.tile([N, N], F32, tag="ep0f")
                    epsum_f = sb.tile([N, 1], F32, tag="epsumf")
                    nc.vector.scalar_tensor_tensor(out=ep0f[:], in0=ep_cur[:], scalar=1.0,
                                                   in1=offd[:], op0=ALU.mult, op1=ALU.mult,
                                                   accum_out=epsum_f[:, 0:1])
                    e_t = sb.tile([N, N], F32, tag="e")
                    esum = sb.tile([N, 1], F32, tag="esum")
                    nc.vector.scalar_tensor_tensor(out=e_t[:], in0=s_full[:], scalar=1.0,
                                                   in1=ep0f[:], op0=ALU.mult, op1=ALU.mult,
                                                   accum_out=esum[:, 0:1])

                    if g > 0:
                        # ---------- top-k scatter mask (torch dim-1 scatter repro) ----------
                        kval = int(N * (1.0 - 0.1 * g))
                        nround = (kval + 7) // 8
                        work = sb.tile([N, N], F32, tag="work")
                        nc.vector.tensor_copy(work[:], e_t[:])
                        idx = sb.tile([N, nround * 8], U32, tag="idx")
                        for t in range(nround):
                            v8 = sb.tile([N, 8], F32, tag="v8")
                            nc.vector.max(out=v8[:], in_=work[:])
                            nc.vector.max_index(out=idx[:, 8 * t:8 * t + 8], in_max=v8[:],
                                                in_values=work[:])
                            nc.vector.match_replace(out=work[:], in_to_replace=v8[:],
                                                    in_values=work[:], imm_value=-1.0)
                        idxf = sb.tile([N, nround * 8], F32, tag="idxf")
                        nc.vector.tensor_copy(idxf[:], idx[:])

                        # counts as PSUM columns: cnt[r, c] = sum_i (idx[i,c]==r)
                        cnt_ps = psB.tile([N, N], F32, tag="z2")
                        cgrp = 4
                        for c0 in range(0, kval, cgrp):
                            cn = min(cgrp, kval - c0)
                            estrip = sb3.tile([N, cgrp * N], BF16, tag="estrip")
                            for ci in range(cn):
                                eng = nc.gpsimd if ci % 3 == 2 else nc.vector
                                eng.tensor_scalar(
                                    out=estrip[:, ci * N:(ci + 1) * N], in0=iota_bf[:],
                                    scalar1=idxf[:, c0 + ci:c0 + ci + 1], scalar2=None,
                                    op0=ALU.is_equal)
                            for ci in range(cn):
                                nc.tensor.matmul(cnt_ps[:, c0 + ci:c0 + ci + 1],
                                                 estrip[:, ci * N:(ci + 1) * N],
                                                 ones_col_bf[:])
                        # e_m = min(cnt,1) * e  (cols >= kval are zero)
                        e_m = sb.tile([N, N], F32, tag="em")
                        nc.vector.memset(e_m[:, kval:N], 0.0)
                        nc.vector.scalar_tensor_tensor(out=e_m[:, 0:kval], in0=cnt_ps[:, 0:kval],
                                                       scalar=1.0, in1=e_t[:, 0:kval],
                                                       op0=ALU.min, op1=ALU.mult,
                                                       accum_out=esum[:, 0:1])
                        e_t = e_m

                    # ---------- l1norm * ep_sum, + eye + 1e-6, row-normalize ----------
                    den = sb.tile([N, 1], F32, tag="den")
                    nc.vector.tensor_scalar_max(den[:], esum[:], 1e-12)
                    inv = sb.tile([N, 1], F32, tag="inv")
                    nc.vector.reciprocal(inv[:], den[:])
                    fac = sb.tile([N, 1], F32, tag="fac")
                    nc.vector.tensor_tensor(out=fac[:], in0=epsum_f[:], in1=inv[:], op=ALU.mult)
                    e3 = sb.tile([N, N], F32, tag="e3")
                    rs2 = sb.tile([N, 1], F32, tag="rs2")
                    nc.vector.scalar_tensor_tensor(out=e3[:], in0=e_t[:], scalar=fac[:, 0:1],
                                                   in1=eyep[:], op0=ALU.mult, op1=ALU.add,
                                                   accum_out=rs2[:, 0:1])
                    inv2 = sb.tile([N, 1], F32, tag="inv2")
                    nc.vector.reciprocal(inv2[:], rs2[:])
                    ep_new = st.tile([N, N], F32, tag=f"epn{b}{g}")
                    nc.vector.tensor_scalar(out=ep_new[:], in0=e3[:], scalar1=inv2[:, 0:1],
                                            scalar2=None, op0=ALU.mult)
                    nc.sync.dma_start(out=out_d[b, g], in_=ep_new[:])
                    EP[b] = ep_new
                return

            ATTN = {}
            QKR = {}

            def attn_qk_phase(b, g):
                vpT = VPT[b]
                qT_ps = psM.tile([N, N], F32, tag="misc")
                nc.tensor.matmul(qT_ps[:], WQT[g][:], vpT[:])
                qT_sb = sb.tile([N, N], F32, tag="qTsb")
                nc.scalar.copy(qT_sb[:], qT_ps[:])
                qTr = st.tile([DK, H * N], F32, tag=f"qTr{b}")
                for h in range(H):
                    nc.sync.dma_start(out=qTr[0:DK, h * N:(h + 1) * N],
                                      in_=qT_sb[h * DK:(h + 1) * DK, :])
                kT_ps = psM.tile([N, N], F32, tag="misc")
                nc.tensor.matmul(kT_ps[:], WKT[g][:], vpT[:])
                kT_sb = sb.tile([N, N], F32, tag="kTsb")
                nc.scalar.copy(kT_sb[:], kT_ps[:])
                kTr = st.tile([DK, H * N], F32, tag=f"kTr{b}")
                for h in range(H):
                    nc.sync.dma_start(out=kTr[0:DK, h * N:(h + 1) * N],
                                      in_=kT_sb[h * DK:(h + 1) * DK, :])
                QKR[b] = (qTr, kTr)

            def attn_phase(b, g):
                vp_cur, ep_cur, vpT = VP[b], EP[b], VPT[b]
                qTr, kTr = QKR[b]
                if True:
                    attn = sb.tile([N, N], F32, tag=f"attn{b}")
                    for h in range(H):
                        lg_ps = psA.tile([N, N], F32, tag="z1")
                        nc.tensor.matmul(lg_ps[:], qTr[:, h * N:(h + 1) * N],
                                         kTr[:, h * N:(h + 1) * N])
                        rmax = sb.tile([N, 1], F32, tag="rmax")
                        nc.vector.reduce_max(rmax[:], lg_ps[:], axis=mybir.AxisListType.X)
                        rmaxs = sb.tile([N, 1], F32, tag="rmaxs")
                        nc.vector.tensor_scalar_mul(rmaxs[:], rmax[:], -0.25)
                        expo = sb.tile([N, N], F32, tag="expo")
                        sume = sb.tile([N, 1], F32, tag="sume")
                        nc.scalar.activation(expo[:], lg_ps[:], AF.Exp,
                                             bias=rmaxs[:, 0:1], scale=0.25,
                                             accum_out=sume[:, 0:1])
                        rsum = sb.tile([N, 1], F32, tag="rsum")
                        nc.vector.reciprocal(rsum[:], sume[:])
                        r16 = sb.tile([N, 1], F32, tag="r16")
                        nc.vector.tensor_scalar_mul(r16[:], rsum[:], 1.0 / 16.0)
                        if h == 0:
                            nc.vector.tensor_scalar(out=attn[:], in0=expo[:],
                                                    scalar1=r16[:, 0:1], scalar2=None,
                                                    op0=ALU.mult)
                        else:
                            nc.vector.scalar_tensor_tensor(out=attn[:], in0=expo[:],
                                                           scalar=r16[:, 0:1], in1=attn[:],
                                                           op0=ALU.mult, op1=ALU.add)
                    ATTN[b] = attn
                return

            def d2p_phase(b, g):
                vp_cur, ep_cur, vpT = VP[b], EP[b], VPT[b]
                ep_new = ep_cur
                attn = ATTN[b]
                if True:
                    # ---------- d2p ----------
                    el0 = sb.tile([N, N], F32, tag="el0")
                    nc.vector.scalar_tensor_tensor(out=el0[:], in0=ep_new[:], scalar=0.5,
                                                   in1=attn[:], op0=ALU.mult, op1=ALU.add)
                    el1 = sb.tile([N, N], F32, tag="el1")
                    lsum = sb.tile([N, 1], F32, tag="lsum")
                    nc.vector.scalar_tensor_tensor(out=el1[:], in0=el0[:], scalar=1.0,
                                                   in1=offd[:], op0=ALU.mult, op1=ALU.mult,
                                                   accum_out=lsum[:, 0:1])
                    lden = sb.tile([N, 1], F32, tag="lden")
                    nc.vector.tensor_scalar_max(lden[:], lsum[:], 1e-12)
                    linv = sb.tile([N, 1], F32, tag="linv")
                    nc.vector.reciprocal(linv[:], lden[:])
                    el = sb.tile([N, N], F32, tag="el")
                    nc.vector.tensor_scalar(out=el[:], in0=el1[:], scalar1=linv[:, 0:1],
                                            scalar2=None, op0=ALU.mult)
                    elT_ps = psM.tile([N, N], F32, tag="misc")
                    nc.tensor.transpose(elT_ps[:], el[:], ident[:])
                    elT = sb.tile([N, N], F32, tag="elT")
                    nc.scalar.copy(elT[:], elT_ps[:])
                    aggrT_ps = psM.tile([N, N], F32, tag="misc")
                    nc.tensor.matmul(aggrT_ps[:], vp_cur[:], elT[:])
                    aggrT = sb.tile([N, N], F32, tag="aggrT")
                    nc.scalar.copy(aggrT[:], aggrT_ps[:])
                    h1d = []
                    for ct in range(2):
                        z1d = psB.tile([N, N], F32, tag="z2")
                        nc.tensor.matmul(z1d[:], DW1T[g][0][ct][:], vpT[:], start=True, stop=False)
                        nc.tensor.matmul(z1d[:], DW1T[g][1][ct][:], aggrT[:], start=False, stop=True)
                        h1dt = sb.tile([N, N], F32, tag=f"h1d{ct}")
                        nc.scalar.activation(h1dt[:], z1d[:], AF.Prelu,
                                             bias=DB1[g][ct], scale=DG1[g][ct],
                                             alpha=0.01)
                        h1d.append(h1dt)
                    z2d = psB.tile([N, N], F32, tag="z2")
                    nc.tensor.matmul(z2d[:], DW2T[g][0][:], h1d[0][:], start=True, stop=False)
                    nc.tensor.matmul(z2d[:], DW2T[g][1][:], h1d[1][:], start=False, stop=True)
                    vpT_new = st.tile([N, N], F32, tag=f"vpTn{b}{g}")
                    nc.scalar.activation(vpT_new[:], z2d[:], AF.Prelu,
                                         bias=DB2[g], scale=DG2[g], alpha=0.01)
                    if g < G - 1:
                        vpn_ps = psM.tile([N, N], F32, tag="misc")
                        nc.tensor.transpose(vpn_ps[:], vpT_new[:], ident[:])
                        vp_new = st.tile([N, D], F32, tag=f"vpn{b}{g}")
                        nc.scalar.copy(vp_new[:], vpn_ps[:])
                        VP[b] = vp_new
                    VPT[b] = vpT_new
                return

            for g in range(G):
                mlp_phase(0, g)
                attn_qk_phase(0, g)
                mlp_phase(1, g)
                attn_qk_phase(1, g)
                post_phase(0, g)
                post_phase(1, g)
                attn_phase(0, g)
                attn_phase(1, g)
                d2p_phase(0, g)
                d2p_phase(1, g)
            # final vp is written transposed ([d, i]); host reassembly transposes back
            for b in range(TASKS_PER_CORE):
                nc.sync.dma_start(out=out_d[b, 4], in_=VPT[b][:])

    nc.compile()
    return nc


_CACHE = {}


def kernel(**inputs):
    if "nc" not in _CACHE:
        _CACHE["nc"] = build_nc()
    nc = _CACHE["nc"]

    arr = {k: np.ascontiguousarray(np.asarray(v, dtype=np.float32)) for k, v in inputs.items()}
    weight_keys = [k for k in arr if k not in ("vp", "ep")]
    in_maps = []
    for c in range(NCORES):
        m = {k: arr[k] for k in weight_keys}
        m["vp"] = arr["vp"][c * TASKS_PER_CORE:(c + 1) * TASKS_PER_CORE]
        m["ep"] = arr["ep"][c * TASKS_PER_CORE:(c + 1) * TASKS_PER_CORE]
        in_maps.append(m)

    res = run_bass_kernel_spmd(nc, in_maps, core_ids=list(range(NCORES)))
    eps = np.empty((G, B, N, N), dtype=np.float32)
    sims = np.empty((G, B, N, N), dtype=np.float32)
    vp_out = np.empty((B, N, D), dtype=np.float32)
    for c in range(NCORES):
        o = res.results[c]["out"]
        for t in range(TASKS_PER_CORE):
            bidx = c * TASKS_PER_CORE + t
            eps[0, bidx] = o[t, 0]
            eps[1, bidx] = o[t, 1]
            sims[0, bidx] = o[t, 2]
            sims[1, bidx] = o[t, 3]
            vp_out[bidx] = o[t, 4].T
    return eps, sims, vp_out
